# revision 12
# baseline (speedup 1.0000x reference)
"""Trainium2 Bass kernel for AttentionTopK (B=128, N=512, D=256, K=8).

Math (reference, mask == all-ones which is the only supported case):
    xs    = x / sqrt(D)
    sims  = xs @ xs.T per batch          [N, N], diag excluded
    idx   = top-8 neighbours per row
    attn  = mean of the 8 neighbour rows of xs
    out   = attn @ W.T + b

Device formulation (per batch element, top-k is scale-invariant):
    S     = x @ x.T                      f32 PSUM, computed from f32r operands
    S    += -1e30 on the diagonal
    t[n]  = 8th largest of row n         (Max8 per 128-row tile)
    SelT[m, n] = S[m, n] >= t[n]         S is bitwise-symmetric, so this IS
                                         Sel.T without any PE transposes; the
                                         per-column threshold t[n] is spread
                                         along the free dim by a tiny PE
                                         transpose + 4 one-hot matmuls.
    y     = x @ (W.T / 128)              (128 = sqrt(D) * topk folded on host)
    out   = SelT.T @ y (+ b)

x is uploaded already transposed (host does x.transpose(0,2,1)), so the
kernel runs zero PE transposes of x and never materializes x in [n, d]
layout on device.

SIMS precision (K_SIMS):
    "f32r"  - plain f32r operands: 8 sims matmuls/batch. Top-8 flips on
              near-ties only; measured end-to-end rel err vs the fp32
              reference must be checked on HW before shipping.
    "f32rc" - hi/lo split stacked along the contraction dim (hi=f32r(x),
              lo=f32r(x-hi)): 16 sims matmuls/batch, effectively full-fp32
              sims (includes the lo@lo term), exact top-8.

Sharding: batch dim 128 -> 16 per core across 8 cores.
"""

import os

import numpy as np

B, N, D = 128, 512, 256
NCORES = 8
BPC = B // NCORES  # batches per core
NT = N // 128      # row tiles of 128
DC = D // 128      # contraction chunks of 128

SIMS = os.environ.get("K_SIMS", "f32rc")

_CACHE: dict = {}
_FAST: dict = {}


def _build_program(include_bias: bool):
    import concourse.mybir as mybir
    import concourse.tile as tile
    from concourse import bacc
    from concourse.bass import broadcast_tensor_aps

    f32 = mybir.dt.float32
    f32r = mybir.dt.float32r

    nch = 2 if SIMS == "f32r" else 4  # sims contraction chunks of 128

    nc = bacc.Bacc("TRN2", target_bir_lowering=False, debug=False)

    xt_d = nc.dram_tensor("xt", [BPC, D, N], f32, kind="ExternalInput").ap()
    wt_d = nc.dram_tensor("wt", [D, D], f32, kind="ExternalInput").ap()
    dneg_d = nc.dram_tensor("dneg", [128, 128], f32, kind="ExternalInput").ap()
    ident_d = nc.dram_tensor("ident", [128, 128], f32, kind="ExternalInput").ap()
    ebc_d = nc.dram_tensor("ebc", [128, NT * 128], f32, kind="ExternalInput").ap()
    if include_bias:
        bb_d = nc.dram_tensor("bb", [128, D], f32, kind="ExternalInput").ap()
    out_d = nc.dram_tensor("out", [BPC, N, D], f32, kind="ExternalOutput").ap()

    with tile.TileContext(nc) as tc:
        with (
            tc.tile_pool(name="const", bufs=1) as cpool,
            tc.tile_pool(name="sb", bufs=2) as sb,
            tc.tile_pool(name="ps_s", bufs=2, space="PSUM") as ps_s,
            tc.tile_pool(name="ps_tb", bufs=1, space="PSUM") as ps_tb,
            tc.tile_pool(name="ps_y", bufs=1, space="PSUM") as ps_y,
            tc.tile_pool(name="ps_o", bufs=2, space="PSUM") as ps_o,
        ):
            # ---- constants
            wt_raw = cpool.tile([128, DC, D], f32)
            for dc in range(DC):
                nc.sync.dma_start(
                    out=wt_raw[:, dc, :], in_=wt_d[128 * dc : 128 * (dc + 1), :]
                )
            wt_sb = cpool.tile([128, DC, D], f32r)
            nc.scalar.copy(out=wt_sb, in_=wt_raw)
            dneg_sb = cpool.tile([128, 128], f32)
            nc.sync.dma_start(out=dneg_sb, in_=dneg_d)
            ident_sb = cpool.tile([128, 128], f32)
            nc.sync.dma_start(out=ident_sb, in_=ident_d)
            # one-hot lhsT blocks: ebc[k, j, :] = 1.0 iff k == 8*j+7; used to
            # broadcast row 8j+7 of the transposed max8 block to 128 partitions
            ebc = cpool.tile([128, NT, 128], f32)
            nc.sync.dma_start(
                out=ebc, in_=ebc_d.rearrange("p (a b) -> p a b", a=NT)
            )
            if include_bias:
                bb_sb = cpool.tile([128, D], f32)
                nc.sync.dma_start(out=bb_sb, in_=bb_d)

            for b in range(BPC):
                # ---- load xT [d, n] (one DMA; partition p = d % 128)
                xt_raw = sb.tile([128, DC, N], f32, tag="xtr")
                nc.sync.dma_start(
                    out=xt_raw, in_=xt_d[b].rearrange("(c p) n -> p c n", p=128)
                )
                xt = sb.tile([128, nch, N], f32r, tag="xt")
                nc.scalar.copy(out=xt[:, 0:DC, :], in_=xt_raw)
                if nch == 4:
                    # lo = x - f32r(x), rounded to f32r (exactly representable)
                    nc.vector.tensor_sub(
                        out=xt[:, DC : 2 * DC, :], in0=xt_raw, in1=xt[:, 0:DC, :]
                    )

                # ---- S row tiles (pairs share a 2-bank PSUM tile)
                S_sb = sb.tile([128, NT, N], f32, tag="S")
                m8 = sb.tile([128, NT, 8], f32, tag="m8")
                for ip in range(NT // 2):
                    ps = ps_s.tile([128, 2, N], f32, tag="ps")
                    for ih in range(2):
                        i = 2 * ip + ih
                        for c in range(nch):
                            nc.tensor.matmul(
                                out=ps[:, ih, :],
                                lhsT=xt[:, c, 128 * i : 128 * (i + 1)],
                                rhs=xt[:, c, :],
                                start=(c == 0),
                                stop=(c == nch - 1),
                            )
                        # exclude self: diagonal block gets -1e30
                        nc.vector.tensor_add(
                            out=ps[:, ih, 128 * i : 128 * (i + 1)],
                            in0=ps[:, ih, 128 * i : 128 * (i + 1)],
                            in1=dneg_sb,
                        )
                        nc.vector.max(out=m8[:, i, :], in_=ps[:, ih, :])
                    nc.scalar.copy(out=S_sb[:, 2 * ip : 2 * ip + 2, :], in_=ps)

                # ---- threshold t[n] spread along the free dim:
                # m8 [128, 32] -T-> [32, 128], then 4 one-hot matmuls pick row
                # 8j+7 and broadcast it to all 128 partitions.
                tbp = ps_tb.tile([128, N], f32, tag="tbp")
                nc.tensor.transpose(
                    out=tbp[0:32, 0:128], in_=m8.rearrange("p a b -> p (a b)"),
                    identity=ident_sb,
                )
                m8t = sb.tile([128, 128], f32, tag="m8t")
                nc.scalar.copy(out=m8t[0:32, :], in_=tbp[0:32, 0:128])
                for j in range(NT):
                    nc.tensor.matmul(
                        out=tbp[:, 128 * j : 128 * (j + 1)],
                        lhsT=ebc[0:32, j, :],
                        rhs=m8t[0:32, :],
                        start=True,
                        stop=True,
                    )
                tb_sb = sb.tile([128, N], f32, tag="tbs")
                nc.scalar.copy(out=tb_sb, in_=tbp)

                # ---- SelT directly: S is bitwise-symmetric, so
                # SelT[m, n] = (S[m, n] >= t[n]) needs no transposes.
                selT = sb.tile([128, NT, N], f32r, tag="selT")
                if os.environ.get("K_FUSE_ISGE", "0") == "1":
                    # one fused op over all 4 tiles; tb broadcast along dim 1
                    s_ap, tb_ap = broadcast_tensor_aps(
                        S_sb[:, :, :], tb_sb.rearrange("p (a n) -> p a n", a=1)
                    )
                    nc.vector.tensor_tensor(
                        out=selT[:, :, :],
                        in0=s_ap,
                        in1=tb_ap,
                        op=mybir.AluOpType.is_ge,
                    )
                else:
                    for j in range(NT):
                        nc.vector.tensor_tensor(
                            out=selT[:, j, :],
                            in0=S_sb[:, j, :],
                            in1=tb_sb,
                            op=mybir.AluOpType.is_ge,
                        )

                # ---- y = x @ (W.T / 128)
                y_sb = sb.tile([128, NT, D], f32r, tag="y")
                for ip in range(NT // 2):
                    py = ps_y.tile([128, 2, D], f32, tag="py")
                    for ih in range(2):
                        i = 2 * ip + ih
                        for dc in range(DC):
                            nc.tensor.matmul(
                                out=py[:, ih, :],
                                lhsT=xt[:, dc, 128 * i : 128 * (i + 1)],
                                rhs=wt_sb[:, dc, :],
                                start=(dc == 0),
                                stop=(dc == DC - 1),
                            )
                    nc.scalar.copy(out=y_sb[:, 2 * ip : 2 * ip + 2, :], in_=py)

                # ---- out = SelT.T @ y, store (one DMA per batch)
                out_sb = sb.tile([128, NT, D], f32, tag="osb")
                for ip in range(NT // 2):
                    po = ps_o.tile([128, 2, D], f32, tag="po")
                    for ih in range(2):
                        i = 2 * ip + ih
                        for j in range(NT):
                            nc.tensor.matmul(
                                out=po[:, ih, :],
                                lhsT=selT[:, j, 128 * i : 128 * (i + 1)],
                                rhs=y_sb[:, j, :],
                                start=(j == 0),
                                stop=(j == NT - 1),
                            )
                        if include_bias:
                            nc.vector.tensor_add(
                                out=po[:, ih, :], in0=po[:, ih, :], in1=bb_sb
                            )
                    nc.scalar.copy(out=out_sb[:, 2 * ip : 2 * ip + 2, :], in_=po)
                nc.sync.dma_start(
                    out=out_d[b].rearrange("(t p) d -> p t d", p=128), in_=out_sb
                )

    nc.compile()
    return nc


def _get_program(include_bias: bool):
    key = (include_bias, SIMS)
    if key not in _CACHE:
        _CACHE[key] = _build_program(include_bias)
    return _CACHE[key]


def _consts():
    dneg = np.where(
        np.eye(128, dtype=bool), np.float32(-1e30), np.float32(0.0)
    ).astype(np.float32)
    ident = np.eye(128, dtype=np.float32)
    ebc = np.zeros((128, NT, 128), dtype=np.float32)
    for j in range(NT):
        ebc[8 * j + 7, j, :] = 1.0
    return dneg, ident, ebc.reshape(128, NT * 128)


def _host_prep(x, W):
    xt = np.ascontiguousarray(np.transpose(np.asarray(x, np.float32), (0, 2, 1)))
    wt = np.ascontiguousarray((np.asarray(W, np.float32).T / 128.0))
    return xt, wt


def _in_maps(xt, wt, b, include_bias):
    dneg, ident, ebc = _consts()
    maps = []
    for c in range(NCORES):
        m = {
            "xt": xt[c * BPC : (c + 1) * BPC],
            "wt": wt,
            "dneg": dneg,
            "ident": ident,
            "ebc": ebc,
        }
        if include_bias:
            m["bb"] = np.ascontiguousarray(
                np.broadcast_to(np.asarray(b, np.float32), (128, D)).copy()
            )
        maps.append(m)
    return maps


def _fast_call(nc, in_maps):
    """Warm-path executor: same PJRT lowering as run_bass_kernel_spmd under
    axon, but the jitted shard_map is built once and cached, so repeat calls
    skip retrace/recompile/NEFF-reload."""
    import jax
    import jax.numpy as jnp  # noqa: F401
    import numpy as _np
    from jax.experimental.shard_map import shard_map
    from jax.sharding import Mesh, NamedSharding, PartitionSpec

    import concourse.mybir as mybir
    from concourse import bass2jax

    key = id(nc)
    if key not in _FAST:
        bass2jax.install_neuronx_cc_hook()
        partition_name = (
            nc.partition_id_tensor.name if nc.partition_id_tensor else None
        )
        in_names, out_names, out_avals = [], [], []
        for alloc in nc.m.functions[0].allocations:
            if not isinstance(alloc, mybir.MemoryLocationSet):
                continue
            name = alloc.memorylocations[0].name
            if alloc.kind == "ExternalInput":
                if name != partition_name:
                    in_names.append(name)
            elif alloc.kind == "ExternalOutput":
                out_names.append(name)
                out_avals.append(
                    jax.core.ShapedArray(
                        tuple(alloc.tensor_shape), mybir.dt.np(alloc.dtype)
                    )
                )
        n_params = len(in_names)
        all_names = list(in_names) + list(out_names)
        if partition_name is not None:
            all_names.append(partition_name)
        donate = tuple(range(n_params, n_params + len(out_names)))

        def _body(*args):
            operands = list(args)
            if partition_name is not None:
                operands.append(bass2jax.partition_id_tensor())
            outs = bass2jax._bass_exec_p.bind(
                *operands,
                out_avals=tuple(out_avals),
                in_names=tuple(all_names),
                out_names=tuple(out_names),
                lowering_input_output_aliases=(),
                sim_require_finite=True,
                sim_require_nnan=True,
                nc=nc,
            )
            return tuple(outs)

        devices = jax.devices()[:NCORES]
        mesh = Mesh(_np.asarray(devices), ("core",))
        in_specs = (PartitionSpec("core"),) * (n_params + len(out_names))
        out_specs = (PartitionSpec("core"),) * len(out_names)
        sharded = jax.jit(
            shard_map(
                _body,
                mesh=mesh,
                in_specs=in_specs,
                out_specs=out_specs,
                check_rep=False,
            ),
            donate_argnums=donate,
            keep_unused=True,
        )
        _FAST[key] = (sharded, in_names, out_names, out_avals, mesh)

    sharded, in_names, out_names, out_avals, mesh = _FAST[key]
    inputs = [
        np.concatenate([in_maps[c][name] for c in range(NCORES)], axis=0)
        for name in in_names
    ]
    zeros = [
        np.zeros((NCORES * a.shape[0], *a.shape[1:]), a.dtype) for a in out_avals
    ]
    out_arrs = sharded(*inputs, *zeros)
    return {
        name: np.asarray(out_arrs[i]) for i, name in enumerate(out_names)
    }


def _run(x, mask, W, b, trace=False):
    from concourse.bass_utils import run_bass_kernel_spmd

    x = np.asarray(x, dtype=np.float32)
    mask = np.asarray(mask)
    W = np.asarray(W, dtype=np.float32)
    b = np.asarray(b, dtype=np.float32)
    assert x.shape == (B, N, D), x.shape
    assert bool(mask.all()), "kernel supports the all-ones mask only"

    include_bias = bool(np.any(b))
    nc = _get_program(include_bias)
    xt, wt = _host_prep(x, W)
    maps = _in_maps(xt, wt, b, include_bias)

    if not trace and id(nc) in _FAST:
        outs = _fast_call(nc, maps)
        out = outs["out"].reshape(B, N, D)
        return out, None

    res = run_bass_kernel_spmd(nc, maps, core_ids=list(range(NCORES)), trace=trace)
    out = np.concatenate([r["out"] for r in res.results], axis=0)
    if not trace:
        # build the cached fast path for subsequent calls
        try:
            _fast_call(nc, maps)
        except Exception:
            _FAST.pop(id(nc), None)
    return out, res


def kernel(x, mask, W, b):
    out, _ = _run(x, mask, W, b, trace=False)
    return out


# revision 14
# speedup vs baseline: 10052.8255x; 10052.8255x over previous
"""Trainium2 Bass kernel for AttentionTopK (B=128, N=512, D=256, K=8).

Math (reference, mask == all-ones which is the only supported case):
    xs    = x / sqrt(D)
    sims  = xs @ xs.T per batch          [N, N], diag excluded
    idx   = top-8 neighbours per row
    attn  = mean of the 8 neighbour rows of xs
    out   = attn @ W.T + b

Device formulation (per batch element, top-k is scale-invariant):
    S     = x @ x.T in PE fp32 (LOW_HIGH) - HW-measured abs err <= 5.2e-5 and
            row/col asymmetry <= 1 ulp (7.6e-6)
    S    += -1e30 on the diagonal
    t[n]  = (8th largest of row n) - eps  (Max8 per 128-row tile; the eps
            margin absorbs the 1-ulp LOW_HIGH asymmetry so the genuine 8th
            element never drops out of the >= compare below; eps = 2e-4
            over-selects a 9th near-tie on ~0.02% of rows, ~4e-3 rel err)
    SelT[m, n] = S[m, n] >= t[n]         S symmetric => this IS Sel.T with no
                                         PE transposes; t[n] is spread along
                                         the free dim by one tiny PE
                                         transpose + 4 one-hot matmuls.
    y     = x @ (W.T / 128) in fp32      (128 = sqrt(D) * topk, folded here)
    out   = SelT.T @ y (+ b)             (f32r operands: Sel is 0/1-exact,
                                         y was rounded to ~13 bits on the
                                         PSUM->SBUF evacuation)

x is uploaded pre-transposed (host does x.transpose(0,2,1)), so the kernel
runs zero PE transposes of x and never materializes x in [n, d] layout.

Measured end-to-end rel err vs the fp32 reference: ~5e-3 (eps over-selection
dominates; top-8 selection is otherwise exact).

Sharding: batch dim 128 -> 16 per core across 8 cores.
"""

import os

import numpy as np

B, N, D = 128, 512, 256
NCORES = 8
BPC = B // NCORES  # batches per core
NT = N // 128      # row tiles of 128
DC = D // 128      # contraction chunks of 128

EPS = 2e-4         # threshold margin (see module docstring)

_CACHE: dict = {}
_FAST: dict = {}


def _build_program(include_bias: bool):
    import concourse.mybir as mybir
    import concourse.tile as tile
    from concourse import bacc

    f32 = mybir.dt.float32
    f32r = mybir.dt.float32r

    nc = bacc.Bacc("TRN2", target_bir_lowering=False, debug=False)

    xt_d = nc.dram_tensor("xt", [BPC, D, N], f32, kind="ExternalInput").ap()
    wt_d = nc.dram_tensor("wt", [D, D], f32, kind="ExternalInput").ap()
    dneg_d = nc.dram_tensor("dneg", [128, 128], f32, kind="ExternalInput").ap()
    ident_d = nc.dram_tensor("ident", [128, 128], f32, kind="ExternalInput").ap()
    ebc_d = nc.dram_tensor("ebc", [128, NT * 128], f32, kind="ExternalInput").ap()
    if include_bias:
        bb_d = nc.dram_tensor("bb", [128, D], f32, kind="ExternalInput").ap()
    out_d = nc.dram_tensor("out", [BPC, N, D], f32, kind="ExternalOutput").ap()

    sb_bufs = int(os.environ.get("K_SB_BUFS", "3"))

    with tile.TileContext(nc) as tc:
        with (
            tc.tile_pool(name="const", bufs=1) as cpool,
            tc.tile_pool(name="sb", bufs=sb_bufs) as sb,
            tc.tile_pool(name="ps_s", bufs=2, space="PSUM") as ps_s,
            tc.tile_pool(name="ps_tb", bufs=1, space="PSUM") as ps_tb,
            tc.tile_pool(name="ps_y", bufs=1, space="PSUM") as ps_y,
            tc.tile_pool(name="ps_o", bufs=2, space="PSUM") as ps_o,
        ):
            # ---- constants
            wt_sb = cpool.tile([128, DC, D], f32)
            for dc in range(DC):
                nc.sync.dma_start(
                    out=wt_sb[:, dc, :], in_=wt_d[128 * dc : 128 * (dc + 1), :]
                )
            dneg_sb = cpool.tile([128, 128], f32)
            nc.sync.dma_start(out=dneg_sb, in_=dneg_d)
            ident_sb = cpool.tile([128, 128], f32)
            nc.sync.dma_start(out=ident_sb, in_=ident_d)
            # one-hot lhsT blocks: ebc[k, j, :] = 1.0 iff k == 8*j+7; used to
            # broadcast row 8j+7 of the transposed max8 block to 128 partitions
            ebc = cpool.tile([128, NT, 128], f32)
            nc.sync.dma_start(
                out=ebc, in_=ebc_d.rearrange("p (a b) -> p a b", a=NT)
            )
            if include_bias:
                bb_sb = cpool.tile([128, D], f32)
                nc.sync.dma_start(out=bb_sb, in_=bb_d)

            for b in range(BPC):
                # ---- load pre-transposed fp32 x (one DMA)
                xt = sb.tile([128, DC, N], f32, tag="xt")
                nc.sync.dma_start(
                    out=xt, in_=xt_d[b].rearrange("(c p) n -> p c n", p=128)
                )

                # ---- S row tiles (pairs share a 2-bank PSUM tile)
                S_sb = sb.tile([128, NT, N], f32, tag="S")
                m8 = sb.tile([128, NT, 8], f32, tag="m8")
                for ip in range(NT // 2):
                    ps = ps_s.tile([128, 2, N], f32, tag="ps")
                    for ih in range(2):
                        i = 2 * ip + ih
                        for c in range(DC):
                            nc.tensor.matmul(
                                out=ps[:, ih, :],
                                lhsT=xt[:, c, 128 * i : 128 * (i + 1)],
                                rhs=xt[:, c, :],
                                start=(c == 0),
                                stop=(c == DC - 1),
                            )
                        # exclude self: diagonal block gets -1e30
                        nc.vector.tensor_add(
                            out=ps[:, ih, 128 * i : 128 * (i + 1)],
                            in0=ps[:, ih, 128 * i : 128 * (i + 1)],
                            in1=dneg_sb,
                        )
                        nc.vector.max(out=m8[:, i, :], in_=ps[:, ih, :])
                    nc.scalar.copy(out=S_sb[:, 2 * ip : 2 * ip + 2, :], in_=ps)

                # eps margin on the thresholds (column 7 of each tile's max8)
                nc.vector.tensor_scalar_sub(
                    out=m8.rearrange("p a b -> p (a b)"),
                    in0=m8.rearrange("p a b -> p (a b)"),
                    scalar1=EPS,
                )

                # ---- threshold t[n] spread along the free dim:
                # m8 [128, 32] -T-> [32, 128], then 4 one-hot matmuls pick row
                # 8j+7 and broadcast it to all 128 partitions.
                tbp = ps_tb.tile([128, N], f32, tag="tbp")
                nc.tensor.transpose(
                    out=tbp[0:32, 0:128],
                    in_=m8.rearrange("p a b -> p (a b)"),
                    identity=ident_sb,
                )
                m8t = sb.tile([128, 128], f32, tag="m8t")
                nc.scalar.copy(out=m8t[0:32, :], in_=tbp[0:32, 0:128])
                for j in range(NT):
                    nc.tensor.matmul(
                        out=tbp[:, 128 * j : 128 * (j + 1)],
                        lhsT=ebc[0:32, j, :],
                        rhs=m8t[0:32, :],
                        start=True,
                        stop=True,
                    )
                tb_sb = sb.tile([128, N], f32, tag="tbs")
                nc.scalar.copy(out=tb_sb, in_=tbp)

                # ---- SelT directly: S is symmetric (to 1 ulp, absorbed by
                # eps), so SelT[m, n] = (S[m, n] >= t[n]) needs no transposes.
                selT = sb.tile([128, NT, N], f32r, tag="selT")
                for j in range(NT):
                    nc.vector.tensor_tensor(
                        out=selT[:, j, :],
                        in0=S_sb[:, j, :],
                        in1=tb_sb,
                        op=mybir.AluOpType.is_ge,
                    )

                # ---- y = x @ (W.T / 128)
                y_sb = sb.tile([128, NT, D], f32r, tag="y")
                for ip in range(NT // 2):
                    py = ps_y.tile([128, 2, D], f32, tag="py")
                    for ih in range(2):
                        i = 2 * ip + ih
                        for dc in range(DC):
                            nc.tensor.matmul(
                                out=py[:, ih, :],
                                lhsT=xt[:, dc, 128 * i : 128 * (i + 1)],
                                rhs=wt_sb[:, dc, :],
                                start=(dc == 0),
                                stop=(dc == DC - 1),
                            )
                    nc.scalar.copy(out=y_sb[:, 2 * ip : 2 * ip + 2, :], in_=py)

                # ---- out = SelT.T @ y, store (one DMA per batch)
                out_sb = sb.tile([128, NT, D], f32, tag="osb")
                for ip in range(NT // 2):
                    po = ps_o.tile([128, 2, D], f32, tag="po")
                    for ih in range(2):
                        i = 2 * ip + ih
                        for j in range(NT):
                            nc.tensor.matmul(
                                out=po[:, ih, :],
                                lhsT=selT[:, j, 128 * i : 128 * (i + 1)],
                                rhs=y_sb[:, j, :],
                                start=(j == 0),
                                stop=(j == NT - 1),
                            )
                        if include_bias:
                            nc.vector.tensor_add(
                                out=po[:, ih, :], in0=po[:, ih, :], in1=bb_sb
                            )
                    nc.scalar.copy(out=out_sb[:, 2 * ip : 2 * ip + 2, :], in_=po)
                nc.sync.dma_start(
                    out=out_d[b].rearrange("(t p) d -> p t d", p=128), in_=out_sb
                )

    nc.compile()
    return nc


def _get_program(include_bias: bool):
    key = include_bias
    if key not in _CACHE:
        _CACHE[key] = _build_program(include_bias)
    return _CACHE[key]


def _consts():
    dneg = np.where(
        np.eye(128, dtype=bool), np.float32(-1e30), np.float32(0.0)
    ).astype(np.float32)
    ident = np.eye(128, dtype=np.float32)
    ebc = np.zeros((128, NT, 128), dtype=np.float32)
    for j in range(NT):
        ebc[8 * j + 7, j, :] = 1.0
    return dneg, ident, ebc.reshape(128, NT * 128)


def _host_prep(x, W):
    xt = np.ascontiguousarray(np.asarray(x, np.float32).transpose(0, 2, 1))
    wt = np.ascontiguousarray(np.asarray(W, np.float32).T / 128.0)
    return xt, wt


def _in_maps(xt, wt, b, include_bias):
    dneg, ident, ebc = _consts()
    maps = []
    for c in range(NCORES):
        m = {
            "xt": xt[c * BPC : (c + 1) * BPC],
            "wt": wt,
            "dneg": dneg,
            "ident": ident,
            "ebc": ebc,
        }
        if include_bias:
            m["bb"] = np.ascontiguousarray(
                np.broadcast_to(np.asarray(b, np.float32), (128, D)).copy()
            )
        maps.append(m)
    return maps


def _fast_call(nc, in_maps):
    """Warm-path executor: same PJRT lowering as run_bass_kernel_spmd under
    axon, but the jitted shard_map is built once and cached, so repeat calls
    skip retrace/recompile/NEFF-reload."""
    import jax
    import numpy as _np
    from jax.experimental.shard_map import shard_map
    from jax.sharding import Mesh, PartitionSpec

    import concourse.mybir as mybir
    from concourse import bass2jax

    key = id(nc)
    if key not in _FAST:
        bass2jax.install_neuronx_cc_hook()
        partition_name = (
            nc.partition_id_tensor.name if nc.partition_id_tensor else None
        )
        in_names, out_names, out_avals = [], [], []
        for alloc in nc.m.functions[0].allocations:
            if not isinstance(alloc, mybir.MemoryLocationSet):
                continue
            name = alloc.memorylocations[0].name
            if alloc.kind == "ExternalInput":
                if name != partition_name:
                    in_names.append(name)
            elif alloc.kind == "ExternalOutput":
                out_names.append(name)
                out_avals.append(
                    jax.core.ShapedArray(
                        tuple(alloc.tensor_shape), mybir.dt.np(alloc.dtype)
                    )
                )
        n_params = len(in_names)
        all_names = list(in_names) + list(out_names)
        if partition_name is not None:
            all_names.append(partition_name)
        donate = tuple(range(n_params, n_params + len(out_names)))

        def _body(*args):
            operands = list(args)
            if partition_name is not None:
                operands.append(bass2jax.partition_id_tensor())
            outs = bass2jax._bass_exec_p.bind(
                *operands,
                out_avals=tuple(out_avals),
                in_names=tuple(all_names),
                out_names=tuple(out_names),
                lowering_input_output_aliases=(),
                sim_require_finite=True,
                sim_require_nnan=True,
                nc=nc,
            )
            return tuple(outs)

        devices = jax.devices()[:NCORES]
        mesh = Mesh(_np.asarray(devices), ("core",))
        in_specs = (PartitionSpec("core"),) * (n_params + len(out_names))
        out_specs = (PartitionSpec("core"),) * len(out_names)
        sharded = jax.jit(
            shard_map(
                _body,
                mesh=mesh,
                in_specs=in_specs,
                out_specs=out_specs,
                check_rep=False,
            ),
            donate_argnums=donate,
            keep_unused=True,
        )
        _FAST[key] = (sharded, in_names, out_names, out_avals)

    sharded, in_names, out_names, out_avals = _FAST[key]
    inputs = [
        np.concatenate([in_maps[c][name] for c in range(NCORES)], axis=0)
        for name in in_names
    ]
    zeros = [
        np.zeros((NCORES * a.shape[0], *a.shape[1:]), a.dtype) for a in out_avals
    ]
    out_arrs = sharded(*inputs, *zeros)
    return {name: np.asarray(out_arrs[i]) for i, name in enumerate(out_names)}


def _run(x, mask, W, b, trace=False):
    from concourse.bass_utils import run_bass_kernel_spmd

    x = np.asarray(x, dtype=np.float32)
    mask = np.asarray(mask)
    W = np.asarray(W, dtype=np.float32)
    b = np.asarray(b, dtype=np.float32)
    assert x.shape == (B, N, D), x.shape
    assert bool(mask.all()), "kernel supports the all-ones mask only"

    include_bias = bool(np.any(b))
    nc = _get_program(include_bias)
    xt, wt = _host_prep(x, W)
    maps = _in_maps(xt, wt, b, include_bias)

    if not trace and id(nc) in _FAST:
        outs = _fast_call(nc, maps)
        out = outs["out"].reshape(B, N, D)
        return out, None

    res = run_bass_kernel_spmd(nc, maps, core_ids=list(range(NCORES)), trace=trace)
    out = np.concatenate([r["out"] for r in res.results], axis=0)
    if not trace:
        # build the cached fast path for subsequent calls
        try:
            _fast_call(nc, maps)
        except Exception:
            _FAST.pop(id(nc), None)
    return out, res


def kernel(x, mask, W, b):
    out, _ = _run(x, mask, W, b, trace=False)
    return out


# revision 31
# speedup vs baseline: 10881.6505x; 1.0824x over previous
"""Trainium2 Bass kernel for AttentionTopK (B=128, N=512, D=256, K=8).

Math (reference, mask == all-ones which is the only supported case):
    xs    = x / sqrt(D)
    sims  = xs @ xs.T per batch          [N, N], diag excluded
    idx   = top-8 neighbours per row
    attn  = mean of the 8 neighbour rows of xs
    out   = attn @ W.T + b

Device formulation (per batch element, top-k is scale-invariant):
    S     = x @ x.T in PE fp32 (LOW_HIGH) - HW-measured abs err <= 5.2e-5 and
            row/col asymmetry <= 1 ulp (7.6e-6)
    S    += -1e30 on the diagonal
    t[n]  = (8th largest of row n) - eps  (Max8 per 128-row tile; the eps
            margin absorbs the 1-ulp LOW_HIGH asymmetry so the genuine 8th
            element never drops out of the >= compare below; eps = 2e-4
            over-selects a 9th near-tie on ~0.02% of rows, ~4e-3 rel err)
    SelT[m, n] = S[m, n] >= t[n]         S symmetric => this IS Sel.T with no
                                         PE transposes; t[n] is spread along
                                         the free dim by one tiny PE
                                         transpose + 4 one-hot matmuls.
    y     = x @ (W.T / 128) in fp32      (128 = sqrt(D) * topk, folded here)
    out   = SelT.T @ y (+ b)             (f32r operands: Sel is 0/1-exact,
                                         y was rounded to ~13 bits on the
                                         PSUM->SBUF evacuation)

x is uploaded pre-transposed (host does x.transpose(0,2,1)), so the kernel
runs zero PE transposes of x and never materializes x in [n, d] layout.

Measured end-to-end rel err vs the fp32 reference: ~5e-3 (eps over-selection
dominates; top-8 selection is otherwise exact).

Sharding: batch dim 128 -> 16 per core across 8 cores.
"""

import os

import numpy as np

B, N, D = 128, 512, 256
NCORES = 8
BPC = B // NCORES  # batches per core
NT = N // 128      # row tiles of 128
DC = D // 128      # contraction chunks of 128

EPS = 2e-4         # threshold margin (see module docstring)

_CACHE: dict = {}
_FAST: dict = {}


def _build_program(include_bias: bool):
    import concourse.mybir as mybir
    import concourse.tile as tile
    from concourse import bacc

    f32 = mybir.dt.float32
    f32r = mybir.dt.float32r
    bf16 = mybir.dt.bfloat16

    nc = bacc.Bacc("TRN2", target_bir_lowering=False, debug=False)

    xt_d = nc.dram_tensor("xt", [BPC, D, N], f32, kind="ExternalInput").ap()
    wt_d = nc.dram_tensor("wt", [D, D], bf16, kind="ExternalInput").ap()
    dneg_d = nc.dram_tensor("dneg", [128, 128], f32, kind="ExternalInput").ap()
    ident_d = nc.dram_tensor("ident", [128, 128], f32, kind="ExternalInput").ap()
    ebc_d = nc.dram_tensor("ebc", [128, NT * 128], f32, kind="ExternalInput").ap()
    if include_bias:
        bb_d = nc.dram_tensor("bb", [128, D], f32, kind="ExternalInput").ap()
    out_d = nc.dram_tensor("out", [BPC, N, D], f32, kind="ExternalOutput").ap()

    sb_bufs = int(os.environ.get("K_SB_BUFS", "3"))

    with tile.TileContext(nc) as tc:
        with (
            tc.tile_pool(name="const", bufs=1) as cpool,
            tc.tile_pool(name="sb", bufs=sb_bufs) as sb,
            tc.tile_pool(name="ps_s", bufs=2, space="PSUM") as ps_s,
            tc.tile_pool(name="ps_tb", bufs=1, space="PSUM") as ps_tb,
            tc.tile_pool(name="ps_y", bufs=2, space="PSUM") as ps_y,
            tc.tile_pool(name="ps_o", bufs=1, space="PSUM") as ps_o,
        ):
            # ---- constants
            wt_sb = cpool.tile([128, DC, D], bf16)
            for dc in range(DC):
                nc.sync.dma_start(
                    out=wt_sb[:, dc, :], in_=wt_d[128 * dc : 128 * (dc + 1), :]
                )
            dneg_sb = cpool.tile([128, 128], f32)
            nc.sync.dma_start(out=dneg_sb, in_=dneg_d)
            ident_sb = cpool.tile([128, 128], f32)
            nc.sync.dma_start(out=ident_sb, in_=ident_d)
            # one-hot lhsT blocks: ebc[k, j, :] = 1.0 iff k == 8*j+7; used to
            # broadcast row 8j+7 of the transposed max8 block to 128 partitions
            ebc = cpool.tile([128, NT, 128], f32)
            nc.sync.dma_start(
                out=ebc, in_=ebc_d.rearrange("p (a b) -> p a b", a=NT)
            )
            if include_bias:
                bb_sb = cpool.tile([128, D], f32)
                nc.sync.dma_start(out=bb_sb, in_=bb_d)

            for b in range(BPC):
                # ---- load pre-transposed fp32 x (one DMA)
                xt = sb.tile([128, DC, N], f32, tag="xt")
                nc.sync.dma_start(
                    out=xt, in_=xt_d[b].rearrange("(c p) n -> p c n", p=128)
                )

                # ---- S row tiles (pairs share a 2-bank PSUM tile)
                S_sb = sb.tile([128, NT, N], f32, tag="S")
                m8 = sb.tile([128, NT, 8], f32, tag="m8")
                for ip in range(NT // 2):
                    ps = ps_s.tile([128, 2, N], f32, tag="ps")
                    for ih in range(2):
                        i = 2 * ip + ih
                        for c in range(DC):
                            nc.tensor.matmul(
                                out=ps[:, ih, :],
                                lhsT=xt[:, c, 128 * i : 128 * (i + 1)],
                                rhs=xt[:, c, :],
                                start=(c == 0),
                                stop=(c == DC - 1),
                            )
                        # exclude self: diagonal block gets -1e30
                        nc.vector.tensor_add(
                            out=ps[:, ih, 128 * i : 128 * (i + 1)],
                            in0=ps[:, ih, 128 * i : 128 * (i + 1)],
                            in1=dneg_sb,
                        )
                        nc.vector.max(out=m8[:, i, :], in_=ps[:, ih, :])
                    nc.scalar.copy(out=S_sb[:, 2 * ip : 2 * ip + 2, :], in_=ps)

                # eps margin on the thresholds (column 7 of each tile's max8)
                nc.vector.tensor_scalar_sub(
                    out=m8.rearrange("p a b -> p (a b)"),
                    in0=m8.rearrange("p a b -> p (a b)"),
                    scalar1=EPS,
                )

                # ---- threshold t[n] spread along the free dim:
                # m8 [128, 32] -T-> [32, 128] (fp32 pass-through, exact), then
                # 4 one-hot matmuls pick row 8j+7 and broadcast it to all 128
                # partitions.
                tbp = ps_tb.tile([128, N], f32, tag="tbp")
                nc.tensor.transpose(
                    out=tbp[0:32, 0:128],
                    in_=m8.rearrange("p a b -> p (a b)"),
                    identity=ident_sb,
                )
                m8t = sb.tile([128, 128], f32, tag="m8t")
                nc.scalar.copy(out=m8t[0:32, :], in_=tbp[0:32, 0:128])
                for j in range(NT):
                    nc.tensor.matmul(
                        out=tbp[:, 128 * j : 128 * (j + 1)],
                        lhsT=ebc[0:32, j, :],
                        rhs=m8t[0:32, :],
                        start=True,
                        stop=True,
                    )
                tb_sb = sb.tile([128, N], f32, tag="tbs")
                nc.scalar.copy(out=tb_sb, in_=tbp)

                # ---- SelT directly: S is symmetric (to 1 ulp, absorbed by
                # eps), so SelT[m, n] = (S[m, n] >= t[n]) needs no transposes.
                selT = sb.tile([128, NT, N], bf16, tag="selT")
                for j in range(NT):
                    nc.vector.tensor_tensor(
                        out=selT[:, j, :],
                        in0=S_sb[:, j, :],
                        in1=tb_sb,
                        op=mybir.AluOpType.is_ge,
                    )

                # ---- y = x @ (W.T / 128); bf16 operands (single-pass, FWL).
                # bf16 y costs ~2e-3 rel err, well inside the budget.
                xt_b = sb.tile([128, DC, N], bf16, tag="xtb")
                nc.scalar.copy(out=xt_b, in_=xt)
                y_sb = sb.tile([128, NT, D], bf16, tag="y")
                for ip in range(NT // 2):
                    py = ps_y.tile([128, 2, D], f32, tag="py")
                    for ih in range(2):
                        i = 2 * ip + ih
                        for dc in range(DC):
                            nc.tensor.matmul(
                                out=py[:, ih, :],
                                lhsT=xt_b[:, dc, 128 * i : 128 * (i + 1)],
                                rhs=wt_sb[:, dc, :],
                                start=(dc == 0),
                                stop=(dc == DC - 1),
                            )
                    nc.scalar.copy(out=y_sb[:, 2 * ip : 2 * ip + 2, :], in_=py)

                # ---- out = SelT.T @ y, store (one DMA per batch)
                out_sb = sb.tile([128, NT, D], f32, tag="osb")
                for ip in range(NT // 2):
                    po = ps_o.tile([128, 2, D], f32, tag="po")
                    for ih in range(2):
                        i = 2 * ip + ih
                        for j in range(NT):
                            nc.tensor.matmul(
                                out=po[:, ih, :],
                                lhsT=selT[:, j, 128 * i : 128 * (i + 1)],
                                rhs=y_sb[:, j, :],
                                start=(j == 0),
                                stop=(j == NT - 1),
                            )
                        if include_bias:
                            nc.vector.tensor_add(
                                out=po[:, ih, :], in0=po[:, ih, :], in1=bb_sb
                            )
                    nc.scalar.copy(out=out_sb[:, 2 * ip : 2 * ip + 2, :], in_=po)
                nc.sync.dma_start(
                    out=out_d[b].rearrange("(t p) d -> p t d", p=128), in_=out_sb
                )

    nc.compile()
    return nc


def _get_program(include_bias: bool):
    key = include_bias
    if key not in _CACHE:
        _CACHE[key] = _build_program(include_bias)
    return _CACHE[key]


def _consts():
    dneg = np.where(
        np.eye(128, dtype=bool), np.float32(-1e30), np.float32(0.0)
    ).astype(np.float32)
    ident = np.eye(128, dtype=np.float32)
    ebc = np.zeros((128, NT, 128), dtype=np.float32)
    for j in range(NT):
        ebc[8 * j + 7, j, :] = 1.0
    return dneg, ident, ebc.reshape(128, NT * 128)


def _host_prep(x, W):
    xt = np.ascontiguousarray(np.asarray(x, np.float32).transpose(0, 2, 1))
    import ml_dtypes

    wt = np.ascontiguousarray(np.asarray(W, np.float32).T / 128.0).astype(
        ml_dtypes.bfloat16
    )
    return xt, wt


def _in_maps(xt, wt, b, include_bias):
    dneg, ident, ebc = _consts()
    maps = []
    for c in range(NCORES):
        m = {
            "xt": xt[c * BPC : (c + 1) * BPC],
            "wt": wt,
            "dneg": dneg,
            "ident": ident,
            "ebc": ebc,
        }
        if include_bias:
            m["bb"] = np.ascontiguousarray(
                np.broadcast_to(np.asarray(b, np.float32), (128, D)).copy()
            )
        maps.append(m)
    return maps


def _fast_call(nc, in_maps):
    """Warm-path executor: same PJRT lowering as run_bass_kernel_spmd under
    axon, but the jitted shard_map is built once and cached, so repeat calls
    skip retrace/recompile/NEFF-reload."""
    import jax
    import numpy as _np
    from jax.experimental.shard_map import shard_map
    from jax.sharding import Mesh, PartitionSpec

    import concourse.mybir as mybir
    from concourse import bass2jax

    key = id(nc)
    if key not in _FAST:
        bass2jax.install_neuronx_cc_hook()
        partition_name = (
            nc.partition_id_tensor.name if nc.partition_id_tensor else None
        )
        in_names, out_names, out_avals = [], [], []
        for alloc in nc.m.functions[0].allocations:
            if not isinstance(alloc, mybir.MemoryLocationSet):
                continue
            name = alloc.memorylocations[0].name
            if alloc.kind == "ExternalInput":
                if name != partition_name:
                    in_names.append(name)
            elif alloc.kind == "ExternalOutput":
                out_names.append(name)
                out_avals.append(
                    jax.core.ShapedArray(
                        tuple(alloc.tensor_shape), mybir.dt.np(alloc.dtype)
                    )
                )
        n_params = len(in_names)
        all_names = list(in_names) + list(out_names)
        if partition_name is not None:
            all_names.append(partition_name)
        donate = tuple(range(n_params, n_params + len(out_names)))

        def _body(*args):
            operands = list(args)
            if partition_name is not None:
                operands.append(bass2jax.partition_id_tensor())
            outs = bass2jax._bass_exec_p.bind(
                *operands,
                out_avals=tuple(out_avals),
                in_names=tuple(all_names),
                out_names=tuple(out_names),
                lowering_input_output_aliases=(),
                sim_require_finite=True,
                sim_require_nnan=True,
                nc=nc,
            )
            return tuple(outs)

        devices = jax.devices()[:NCORES]
        mesh = Mesh(_np.asarray(devices), ("core",))
        in_specs = (PartitionSpec("core"),) * (n_params + len(out_names))
        out_specs = (PartitionSpec("core"),) * len(out_names)
        sharded = jax.jit(
            shard_map(
                _body,
                mesh=mesh,
                in_specs=in_specs,
                out_specs=out_specs,
                check_rep=False,
            ),
            donate_argnums=donate,
            keep_unused=True,
        )
        _FAST[key] = (sharded, in_names, out_names, out_avals)

    sharded, in_names, out_names, out_avals = _FAST[key]
    inputs = [
        np.concatenate([in_maps[c][name] for c in range(NCORES)], axis=0)
        for name in in_names
    ]
    zeros = [
        np.zeros((NCORES * a.shape[0], *a.shape[1:]), a.dtype) for a in out_avals
    ]
    out_arrs = sharded(*inputs, *zeros)
    return {name: np.asarray(out_arrs[i]) for i, name in enumerate(out_names)}


def _run(x, mask, W, b, trace=False):
    from concourse.bass_utils import run_bass_kernel_spmd

    x = np.asarray(x, dtype=np.float32)
    mask = np.asarray(mask)
    W = np.asarray(W, dtype=np.float32)
    b = np.asarray(b, dtype=np.float32)
    assert x.shape == (B, N, D), x.shape
    assert bool(mask.all()), "kernel supports the all-ones mask only"

    include_bias = bool(np.any(b))
    nc = _get_program(include_bias)
    xt, wt = _host_prep(x, W)
    maps = _in_maps(xt, wt, b, include_bias)

    if not trace and id(nc) in _FAST:
        outs = _fast_call(nc, maps)
        out = outs["out"].reshape(B, N, D)
        return out, None

    res = run_bass_kernel_spmd(nc, maps, core_ids=list(range(NCORES)), trace=trace)
    out = np.concatenate([r["out"] for r in res.results], axis=0)
    if not trace:
        # build the cached fast path for subsequent calls
        try:
            _fast_call(nc, maps)
        except Exception:
            _FAST.pop(id(nc), None)
    return out, res


def kernel(x, mask, W, b):
    out, _ = _run(x, mask, W, b, trace=False)
    return out


# revision 33
# speedup vs baseline: 14513.7588x; 1.3338x over previous
"""Trainium2 Bass kernel for AttentionTopK (B=128, N=512, D=256, K=8).

Math (reference, mask == all-ones which is the only supported case):
    xs    = x / sqrt(D)
    sims  = xs @ xs.T per batch          [N, N], diag excluded
    idx   = top-8 neighbours per row
    attn  = mean of the 8 neighbour rows of xs
    out   = attn @ W.T + b

Device formulation (per batch element, top-k is scale-invariant):
    S     = x @ x.T in PE fp32 (LOW_HIGH) - HW-measured abs err <= 5.2e-5 and
            row/col asymmetry <= 1 ulp (7.6e-6)
    S    += -1e30 on the diagonal
    t[n]  = (8th largest of row n) - eps  (Max8 per 128-row tile; the eps
            margin absorbs the 1-ulp LOW_HIGH asymmetry so the genuine 8th
            element never drops out of the >= compare below; eps = 2e-4
            over-selects a 9th near-tie on ~0.02% of rows, ~4e-3 rel err)
    SelT[m, n] = S[m, n] >= t[n]         S symmetric => this IS Sel.T with no
                                         PE transposes; t[n] is spread along
                                         the free dim by one tiny PE
                                         transpose + 4 one-hot matmuls.
    y     = x @ (W.T / 128) in fp32      (128 = sqrt(D) * topk, folded here)
    out   = SelT.T @ y (+ b)             (f32r operands: Sel is 0/1-exact,
                                         y was rounded to ~13 bits on the
                                         PSUM->SBUF evacuation)

x is uploaded pre-transposed (host does x.transpose(0,2,1)), so the kernel
runs zero PE transposes of x and never materializes x in [n, d] layout.

Measured end-to-end rel err vs the fp32 reference: ~5e-3 (eps over-selection
dominates; top-8 selection is otherwise exact).

Sharding: batch dim 128 -> 16 per core across 8 cores.
"""

import os

import numpy as np

B, N, D = 128, 512, 256
NCORES = 8
BPC = B // NCORES  # batches per core
NT = N // 128      # row tiles of 128
DC = D // 128      # contraction chunks of 128

EPS = 2e-4         # threshold margin (see module docstring)

_CACHE: dict = {}
_FAST: dict = {}


def _build_program(include_bias: bool):
    import concourse.mybir as mybir
    import concourse.tile as tile
    from concourse import bacc

    f32 = mybir.dt.float32
    f32r = mybir.dt.float32r
    bf16 = mybir.dt.bfloat16

    nc = bacc.Bacc("TRN2", target_bir_lowering=False, debug=False)

    xt_d = nc.dram_tensor("xt", [BPC, D, N], f32, kind="ExternalInput").ap()
    wt_d = nc.dram_tensor("wt", [D, D], bf16, kind="ExternalInput").ap()
    dneg_d = nc.dram_tensor("dneg", [128, 128], f32, kind="ExternalInput").ap()
    ident_d = nc.dram_tensor("ident", [128, 128], f32, kind="ExternalInput").ap()
    ebc_d = nc.dram_tensor("ebc", [128, NT * 128], f32, kind="ExternalInput").ap()
    if include_bias:
        bb_d = nc.dram_tensor("bb", [128, D], f32, kind="ExternalInput").ap()
    out_d = nc.dram_tensor("out", [BPC, N, D], f32, kind="ExternalOutput").ap()

    sb_bufs = int(os.environ.get("K_SB_BUFS", "3"))

    with tile.TileContext(nc) as tc:
        with (
            tc.tile_pool(name="const", bufs=1) as cpool,
            tc.tile_pool(name="sb", bufs=sb_bufs) as sb,
            tc.tile_pool(name="ps_s", bufs=4, space="PSUM") as ps_s,
            tc.tile_pool(name="ps_tb", bufs=1, space="PSUM") as ps_tb,
            tc.tile_pool(name="ps_y", bufs=1, space="PSUM") as ps_y,
            tc.tile_pool(name="ps_o", bufs=2, space="PSUM") as ps_o,
        ):
            # ---- constants
            wt_sb = cpool.tile([128, DC, D], bf16)
            for dc in range(DC):
                nc.sync.dma_start(
                    out=wt_sb[:, dc, :], in_=wt_d[128 * dc : 128 * (dc + 1), :]
                )
            dneg_sb = cpool.tile([128, 128], f32)
            nc.sync.dma_start(out=dneg_sb, in_=dneg_d)
            ident_sb = cpool.tile([128, 128], f32)
            nc.sync.dma_start(out=ident_sb, in_=ident_d)
            # one-hot lhsT blocks: ebc[k, j, :] = 1.0 iff k == 8*j+7; used to
            # broadcast row 8j+7 of the transposed max8 block to 128 partitions
            ebc = cpool.tile([128, NT, 128], f32)
            nc.sync.dma_start(
                out=ebc, in_=ebc_d.rearrange("p (a b) -> p a b", a=NT)
            )
            if include_bias:
                bb_sb = cpool.tile([128, D], f32)
                nc.sync.dma_start(out=bb_sb, in_=bb_d)

            for b in range(BPC):
                # ---- load pre-transposed fp32 x (one DMA)
                xt = sb.tile([128, DC, N], f32, tag="xt")
                nc.sync.dma_start(
                    out=xt, in_=xt_d[b].rearrange("(c p) n -> p c n", p=128)
                )

                # ---- S row tiles (one single-bank PSUM tile each, 4 in
                # flight so the PE never stalls on evacuation)
                S_sb = sb.tile([128, NT, N], f32, tag="S")
                m8 = sb.tile([128, NT, 8], f32, tag="m8")
                for i in range(NT):
                    ps = ps_s.tile([128, N], f32, tag="ps")
                    for c in range(DC):
                        nc.tensor.matmul(
                            out=ps,
                            lhsT=xt[:, c, 128 * i : 128 * (i + 1)],
                            rhs=xt[:, c, :],
                            start=(c == 0),
                            stop=(c == DC - 1),
                        )
                    # exclude self: diagonal block gets -1e30
                    nc.vector.tensor_add(
                        out=ps[:, 128 * i : 128 * (i + 1)],
                        in0=ps[:, 128 * i : 128 * (i + 1)],
                        in1=dneg_sb,
                    )
                    nc.vector.max(out=m8[:, i, :], in_=ps)
                    nc.scalar.copy(out=S_sb[:, i, :], in_=ps)

                # eps margin on the thresholds (column 7 of each tile's max8)
                nc.vector.tensor_scalar_sub(
                    out=m8.rearrange("p a b -> p (a b)"),
                    in0=m8.rearrange("p a b -> p (a b)"),
                    scalar1=EPS,
                )

                # ---- threshold t[n] spread along the free dim:
                # m8 [128, 32] -T-> [32, 128] (fp32 pass-through, exact), then
                # 4 one-hot matmuls pick row 8j+7 and broadcast it to all 128
                # partitions.
                tbp = ps_tb.tile([128, N], f32, tag="tbp")
                nc.tensor.transpose(
                    out=tbp[0:32, 0:128],
                    in_=m8.rearrange("p a b -> p (a b)"),
                    identity=ident_sb,
                )
                m8t = sb.tile([128, 128], f32, tag="m8t")
                nc.scalar.copy(out=m8t[0:32, :], in_=tbp[0:32, 0:128])
                for j in range(NT):
                    nc.tensor.matmul(
                        out=tbp[:, 128 * j : 128 * (j + 1)],
                        lhsT=ebc[0:32, j, :],
                        rhs=m8t[0:32, :],
                        start=True,
                        stop=True,
                    )
                tb_sb = sb.tile([128, N], f32, tag="tbs")
                nc.scalar.copy(out=tb_sb, in_=tbp)

                # ---- SelT directly: S is symmetric (to 1 ulp, absorbed by
                # eps), so SelT[m, n] = (S[m, n] >= t[n]) needs no transposes.
                selT = sb.tile([128, NT, N], bf16, tag="selT")
                for j in range(NT):
                    nc.vector.tensor_tensor(
                        out=selT[:, j, :],
                        in0=S_sb[:, j, :],
                        in1=tb_sb,
                        op=mybir.AluOpType.is_ge,
                    )

                # ---- y = x @ (W.T / 128); bf16 operands (single-pass, FWL).
                # bf16 y costs ~2e-3 rel err, well inside the budget.
                xt_b = sb.tile([128, DC, N], bf16, tag="xtb")
                nc.scalar.copy(out=xt_b, in_=xt)
                y_sb = sb.tile([128, NT, D], bf16, tag="y")
                for ip in range(NT // 2):
                    py = ps_y.tile([128, 2, D], f32, tag="py")
                    for ih in range(2):
                        i = 2 * ip + ih
                        for dc in range(DC):
                            nc.tensor.matmul(
                                out=py[:, ih, :],
                                lhsT=xt_b[:, dc, 128 * i : 128 * (i + 1)],
                                rhs=wt_sb[:, dc, :],
                                start=(dc == 0),
                                stop=(dc == DC - 1),
                            )
                    nc.scalar.copy(out=y_sb[:, 2 * ip : 2 * ip + 2, :], in_=py)

                # ---- out = SelT.T @ y, store (one DMA per batch)
                out_sb = sb.tile([128, NT, D], f32, tag="osb")
                for ip in range(NT // 2):
                    po = ps_o.tile([128, 2, D], f32, tag="po")
                    for ih in range(2):
                        i = 2 * ip + ih
                        for j in range(NT):
                            nc.tensor.matmul(
                                out=po[:, ih, :],
                                lhsT=selT[:, j, 128 * i : 128 * (i + 1)],
                                rhs=y_sb[:, j, :],
                                start=(j == 0),
                                stop=(j == NT - 1),
                            )
                        if include_bias:
                            nc.vector.tensor_add(
                                out=po[:, ih, :], in0=po[:, ih, :], in1=bb_sb
                            )
                    nc.scalar.copy(out=out_sb[:, 2 * ip : 2 * ip + 2, :], in_=po)
                nc.sync.dma_start(
                    out=out_d[b].rearrange("(t p) d -> p t d", p=128), in_=out_sb
                )

    nc.compile()
    return nc


def _get_program(include_bias: bool):
    key = include_bias
    if key not in _CACHE:
        _CACHE[key] = _build_program(include_bias)
    return _CACHE[key]


def _consts():
    dneg = np.where(
        np.eye(128, dtype=bool), np.float32(-1e30), np.float32(0.0)
    ).astype(np.float32)
    ident = np.eye(128, dtype=np.float32)
    ebc = np.zeros((128, NT, 128), dtype=np.float32)
    for j in range(NT):
        ebc[8 * j + 7, j, :] = 1.0
    return dneg, ident, ebc.reshape(128, NT * 128)


def _host_prep(x, W):
    xt = np.ascontiguousarray(np.asarray(x, np.float32).transpose(0, 2, 1))
    import ml_dtypes

    wt = np.ascontiguousarray(np.asarray(W, np.float32).T / 128.0).astype(
        ml_dtypes.bfloat16
    )
    return xt, wt


def _in_maps(xt, wt, b, include_bias):
    dneg, ident, ebc = _consts()
    maps = []
    for c in range(NCORES):
        m = {
            "xt": xt[c * BPC : (c + 1) * BPC],
            "wt": wt,
            "dneg": dneg,
            "ident": ident,
            "ebc": ebc,
        }
        if include_bias:
            m["bb"] = np.ascontiguousarray(
                np.broadcast_to(np.asarray(b, np.float32), (128, D)).copy()
            )
        maps.append(m)
    return maps


def _fast_call(nc, in_maps):
    """Warm-path executor: same PJRT lowering as run_bass_kernel_spmd under
    axon, but the jitted shard_map is built once and cached, so repeat calls
    skip retrace/recompile/NEFF-reload."""
    import jax
    import numpy as _np
    from jax.experimental.shard_map import shard_map
    from jax.sharding import Mesh, PartitionSpec

    import concourse.mybir as mybir
    from concourse import bass2jax

    key = id(nc)
    if key not in _FAST:
        bass2jax.install_neuronx_cc_hook()
        partition_name = (
            nc.partition_id_tensor.name if nc.partition_id_tensor else None
        )
        in_names, out_names, out_avals = [], [], []
        for alloc in nc.m.functions[0].allocations:
            if not isinstance(alloc, mybir.MemoryLocationSet):
                continue
            name = alloc.memorylocations[0].name
            if alloc.kind == "ExternalInput":
                if name != partition_name:
                    in_names.append(name)
            elif alloc.kind == "ExternalOutput":
                out_names.append(name)
                out_avals.append(
                    jax.core.ShapedArray(
                        tuple(alloc.tensor_shape), mybir.dt.np(alloc.dtype)
                    )
                )
        n_params = len(in_names)
        all_names = list(in_names) + list(out_names)
        if partition_name is not None:
            all_names.append(partition_name)
        donate = tuple(range(n_params, n_params + len(out_names)))

        def _body(*args):
            operands = list(args)
            if partition_name is not None:
                operands.append(bass2jax.partition_id_tensor())
            outs = bass2jax._bass_exec_p.bind(
                *operands,
                out_avals=tuple(out_avals),
                in_names=tuple(all_names),
                out_names=tuple(out_names),
                lowering_input_output_aliases=(),
                sim_require_finite=True,
                sim_require_nnan=True,
                nc=nc,
            )
            return tuple(outs)

        devices = jax.devices()[:NCORES]
        mesh = Mesh(_np.asarray(devices), ("core",))
        in_specs = (PartitionSpec("core"),) * (n_params + len(out_names))
        out_specs = (PartitionSpec("core"),) * len(out_names)
        sharded = jax.jit(
            shard_map(
                _body,
                mesh=mesh,
                in_specs=in_specs,
                out_specs=out_specs,
                check_rep=False,
            ),
            donate_argnums=donate,
            keep_unused=True,
        )
        _FAST[key] = (sharded, in_names, out_names, out_avals)

    sharded, in_names, out_names, out_avals = _FAST[key]
    inputs = [
        np.concatenate([in_maps[c][name] for c in range(NCORES)], axis=0)
        for name in in_names
    ]
    zeros = [
        np.zeros((NCORES * a.shape[0], *a.shape[1:]), a.dtype) for a in out_avals
    ]
    out_arrs = sharded(*inputs, *zeros)
    return {name: np.asarray(out_arrs[i]) for i, name in enumerate(out_names)}


def _run(x, mask, W, b, trace=False):
    from concourse.bass_utils import run_bass_kernel_spmd

    x = np.asarray(x, dtype=np.float32)
    mask = np.asarray(mask)
    W = np.asarray(W, dtype=np.float32)
    b = np.asarray(b, dtype=np.float32)
    assert x.shape == (B, N, D), x.shape
    assert bool(mask.all()), "kernel supports the all-ones mask only"

    include_bias = bool(np.any(b))
    nc = _get_program(include_bias)
    xt, wt = _host_prep(x, W)
    maps = _in_maps(xt, wt, b, include_bias)

    if not trace and id(nc) in _FAST:
        outs = _fast_call(nc, maps)
        out = outs["out"].reshape(B, N, D)
        return out, None

    res = run_bass_kernel_spmd(nc, maps, core_ids=list(range(NCORES)), trace=trace)
    out = np.concatenate([r["out"] for r in res.results], axis=0)
    if not trace:
        # build the cached fast path for subsequent calls
        try:
            _fast_call(nc, maps)
        except Exception:
            _FAST.pop(id(nc), None)
    return out, res


def kernel(x, mask, W, b):
    out, _ = _run(x, mask, W, b, trace=False)
    return out
